# revision 11
# baseline (speedup 1.0000x reference)
"""AttentionBlock (GroupNorm + 1x1-conv QKV + spatial attention + 1x1-conv out
+ skip) on 8 Trainium2 NeuronCores.

Sharding: data-parallel over batch. B=16 -> 2 batches per core, weights
replicated, no collectives. Each core runs the same NEFF on its own batch
slice; the host gathers by concatenation.

v3 design (vs the q/k/v baseline):
  * Host folds the 1x1 convs:  M1 = W_q^T W_k  and  M2 = W_o W_v, so
        scores = xn^T M1 xn          (one projection t = M1 xn instead of q,k)
        out    = (M2 xn) attn^T + (W_o b_v + b_o)   (no separate v / proj_out)
    The bias fold is exact because softmax rows sum to 1.
  * Scores are computed TRANSPOSED ([key m on partitions, query n free]):
    kills all PE transposes and the attn normalization pass. The softmax
    denominator Z comes from a ones-stationary matmul over exp(scores^T);
    normalization happens once, fused into the output evacuation
    (out = outU * (1/Z)[n] + skip).
  * All five big matmul groups run fp8e4 DoubleRow (2 MACs/cell/cycle);
    exp() is biased into fp8 range (the fixed e^-SHIFT cancels in Z).
  * Both batches' GroupNorm runs up front (vectorized across channel
    chunks) so the ACT table only swaps once into Exp and the PE never
    stalls mid-kernel on a norm chain.
  * Engine budget: PE ~53us; ACT = exp + all PSUM->SBUF copies (fast at
    ~250ns/tile); DVE = reduces + zr chain + out normalize; Pool = GN
    apply + skip-add.

Layouts on chip (partition dim first):
  channels  c = 128*ct + p   (ct in 0..3)
  spatial   n = 128*mb + p   (mb in 0..7)
  x             [128, 4, 1024]  f32   ([c_part, ct, n])
  xn, t         [128, 4, 1024]  fp8   (t indexed [c_q, m])
  ut            [128, 8, 512]   fp8   ([m_part, mb, c_out])
  E = exp(S^T)  [128, 8, 1024]  fp8   ([m_part, mb, n])
  zr = 1/Z      [128, 2, 512]   f32   (Z broadcast over partitions)
"""

import os
import numpy as np

B, C, H, W = 16, 512, 32, 32
HW = H * W            # 1024
BL = 2                # batches per core
NCORES = 8
CT = C // 128         # 4 channel chunks
NBLK = HW // 128      # 8 spatial blocks
GSIZE = 16            # channels per group
GSLOT = 128 // GSIZE  # 8 groups per channel chunk
CNT = GSIZE * HW      # elements per group (16384)
EPS = 1e-5
INVSQ = float(1.0 / np.sqrt(np.float32(C)))
# exp(score/sqrt(C) - SHIFT): keeps exp output in fp8e4m3's range
# (max |score/sqrt(C)| ~ 6.2 -> exp <= ~110 < 240). e^-SHIFT cancels in Z.
SHIFT = float(os.environ.get("K_SHIFT", "1.5"))
# "fp8": DoubleRow fp8 for all big matmuls. "bf16": same structure, bf16.
V2DT = os.environ.get("K_V2DT", "fp8")
NWARM = int(os.environ.get("K_NWARM", "12"))

_CACHE = {}


def _build_program(need_bias):
    import concourse.bacc as bacc
    import concourse.tile as tile
    from concourse import mybir

    F32 = mybir.dt.float32
    Alu = mybir.AluOpType
    Act = mybir.ActivationFunctionType
    Ax = mybir.AxisListType
    BF16 = mybir.dt.bfloat16
    FP8 = V2DT == "fp8"
    CDT = mybir.dt.float8e4 if FP8 else BF16
    DR = mybir.MatmulPerfMode.DoubleRow if FP8 else None
    KSTEP = 2 if FP8 else 1   # kc contraction per matmul (DoubleRow pairs)
    NK = CT // KSTEP
    NM = NBLK // KSTEP

    nc = bacc.Bacc("TRN2", target_bir_lowering=False, debug=False)

    x_d = nc.dram_tensor("x", [BL, C, HW], F32, kind="ExternalInput")
    m1_d = nc.dram_tensor("m1t", [C, C], CDT, kind="ExternalInput")
    m2_d = nc.dram_tensor("m2t", [C, C], CDT, kind="ExternalInput")
    gam_d = nc.dram_tensor("gamma_t", [128, CT], F32, kind="ExternalInput")
    bet_d = nc.dram_tensor("beta_t", [128, CT], F32, kind="ExternalInput")
    idn_dn_d = nc.dram_tensor("ind_dn", [128, GSLOT], F32, kind="ExternalInput")
    idn_up_d = nc.dram_tensor("ind_up", [GSLOT, 128], F32, kind="ExternalInput")
    if need_bias:
        c1_d = nc.dram_tensor("c1_t", [128, CT], F32, kind="ExternalInput")
        bf_d = nc.dram_tensor("bf_t", [128, CT], F32, kind="ExternalInput")
        wr_d = nc.dram_tensor("wr_t", [128, CT], CDT, kind="ExternalInput")
    out_d = nc.dram_tensor("out", [BL, C, HW], F32, kind="ExternalOutput")

    with tile.TileContext(nc) as tc:
        with (
            tc.tile_pool(name="consts", bufs=1) as cp,
            tc.tile_pool(name="work", bufs=1) as wp,
            tc.tile_pool(name="psum", bufs=1, space="PSUM") as pp,
        ):
            # ---- PE warm-up: the HAM clock gate holds the PE at 1.2 GHz
            # until it sees ~3.4us of sustained matmul activity. Burn the
            # DMA + GroupNorm lead-in on throwaway matmuls.
            warm = cp.tile([128, 512], BF16, name="warm", tag="warm")
            nc.gpsimd.memset(warm[:], 1.0)
            warm_ps = pp.tile([128, 512], F32, name="warm_ps", tag="warm_ps", bufs=1)
            for _ in range(NWARM):
                nc.tensor.matmul(warm_ps[:], warm[:, 0:128], warm[:], start=True, stop=True)

            # ---- small constants on the SWDGE queue ----
            ind_dn = cp.tile([128, GSLOT], F32, name="ind_dn", tag="ind_dn")
            nc.gpsimd.dma_start(ind_dn[:], idn_dn_d[:])
            ind_up = cp.tile([GSLOT, 128], F32, name="ind_up", tag="ind_up")
            nc.gpsimd.dma_start(ind_up[:], idn_up_d[:])
            gam = cp.tile([128, CT], F32, name="gam", tag="gam")
            nc.gpsimd.dma_start(gam[:], gam_d[:])
            bet = cp.tile([128, CT], F32, name="bet", tag="bet")
            nc.gpsimd.dma_start(bet[:], bet_d[:])
            if need_bias:
                c1 = cp.tile([128, CT], F32, name="c1", tag="c1")
                nc.gpsimd.dma_start(c1[:], c1_d[:])
                b_f = cp.tile([128, CT], F32, name="b_f", tag="b_f")
                nc.gpsimd.dma_start(b_f[:], bf_d[:])
                wr = cp.tile([128, CT], CDT, name="wr", tag="wr")
                nc.gpsimd.dma_start(wr[:], wr_d[:])
            ones_dr = cp.tile([128, KSTEP, 16], CDT, name="ones_dr", tag="ones_dr")
            nc.gpsimd.memset(ones_dr[:], 1.0)
            ones_bc = cp.tile([1, 128], BF16, name="ones_bc", tag="ones_bc")
            nc.gpsimd.memset(ones_bc[:], 1.0)
            ebias = cp.tile([128, 1], F32, name="ebias", tag="ebias")
            nc.gpsimd.memset(ebias[:], -SHIFT)

            st = [dict() for _ in range(BL)]

            def load_x(b):
                s = st[b]
                s["x"] = wp.tile([128, CT, HW], F32, name=f"x{b}", tag="x", bufs=2)
                x_r = x_d[b].rearrange("(ct p) n -> p ct n", p=128)
                for ct in range(CT):
                    nc.sync.dma_start(s["x"][:, ct, :], x_r[:, ct, :])

            def gn_stats(b):
                # per-partition sum (DVE, one op across all chunks) and sum of
                # squares: ACT Square+accum for batch 0 (ACT is idle early),
                # DVE tensor_tensor_reduce for batch 1 (keeps the ACT FIFO
                # free for batch-0 evacuations).
                s = st[b]
                s["ssum"] = wp.tile([128, CT, 2], F32, name=f"ssum{b}", tag="ssum", bufs=2)
                scr = wp.tile([128, HW], F32, name=f"scr{b}", tag="scr", bufs=2)
                nc.vector.tensor_reduce(
                    out=s["ssum"][:, :, 0:1], in_=s["x"][:], axis=Ax.X, op=Alu.add
                )
                for ct in range(CT):
                    nc.scalar.activation(
                        scr[:], s["x"][:, ct, :], Act.Square,
                        accum_out=s["ssum"][:, ct, 1:2],
                    )

            def gn_chain(b):
                # one indicator matmul for all chunks -> [GSLOT, CT, 2] group
                # stats; mean/rstd chain vectorized; broadcast back; fused
                # per-chunk apply on Pool writing xn in the compute dtype.
                s = st[b]
                s["xn"] = wp.tile([128, CT, HW], CDT, name=f"xn{b}", tag="xn", bufs=2)
                ab = wp.tile([128, 2 * CT], F32, name=f"ab{b}", tag="ab", bufs=2)
                s["ab"] = ab
                ps_g = pp.tile([GSLOT, CT, 2], F32, name=f"psg{b}", tag="gbc", bufs=1)
                nc.tensor.matmul(ps_g[:], ind_dn[:], s["ssum"][:], start=True, stop=True)
                m_r = wp.tile([GSLOT, CT, 2], F32, name=f"mr{b}", tag="mr", bufs=2)
                t2 = wp.tile([GSLOT, CT, 1], F32, name=f"t2{b}", tag="t2", bufs=2)
                nc.scalar.mul(m_r[:, :, 0:1], ps_g[:, :, 0:1], 1.0 / CNT)   # mean
                nc.scalar.mul(t2[:], ps_g[:, :, 1:2], 1.0 / CNT)            # E[x^2]
                nc.vector.tensor_mul(m_r[:, :, 1:2], m_r[:, :, 0:1], m_r[:, :, 0:1])
                nc.vector.tensor_sub(t2[:], t2[:], m_r[:, :, 1:2])
                nc.vector.tensor_scalar_add(t2[:], t2[:], EPS)
                nc.scalar.activation(t2[:], t2[:], Act.Sqrt)
                nc.vector.reciprocal(m_r[:, :, 1:2], t2[:])                 # rstd
                ps_bc = pp.tile([128, CT, 2], F32, name=f"psbc{b}", tag="gbc", bufs=1)
                nc.tensor.matmul(ps_bc[:], ind_up[:], m_r[:], start=True, stop=True)
                nc.vector.tensor_mul(ab[:, 0:CT], ps_bc[:, :, 1], gam[:])
                nc.vector.tensor_mul(ab[:, CT : 2 * CT], ps_bc[:, :, 0], ab[:, 0:CT])
                nc.vector.tensor_sub(ab[:, CT : 2 * CT], bet[:], ab[:, CT : 2 * CT])
                for ct in range(CT):
                    nc.gpsimd.tensor_scalar(
                        out=s["xn"][:, ct, :], in0=s["x"][:, ct, :],
                        scalar1=ab[:, ct : ct + 1], scalar2=ab[:, CT + ct : CT + ct + 1],
                        op0=Alu.mult, op1=Alu.add,
                    )

            def mm_k(ps, lhs_fn, rhs_fn, nk):
                for k in range(nk):
                    nc.tensor.matmul(
                        ps[:], lhs_fn(k), rhs_fn(k),
                        start=(k == 0), stop=(k == nk - 1),
                        perf_mode=DR,
                    )

            def ksl(t, k, lo, hi):
                return t[:, KSTEP * k : KSTEP * (k + 1), lo:hi]

            def t_mm(b, oc, nh):
                # t[:, oc, nh-half] = (M1 xn)[oc-chunk, half]  (+ c1 if biased)
                s = st[b]
                if "t" not in s:
                    s["t"] = wp.tile([128, CT, HW], CDT, name=f"t{b}", tag="t", bufs=2)
                ps = pp.tile([128, 512], F32, name=f"pt{b}_{oc}_{nh}", tag="mm", bufs=5)
                mm_k(ps,
                     lambda k: ksl(m1, k, oc * 128, (oc + 1) * 128),
                     lambda k: ksl(s["xn"], k, nh * 512, (nh + 1) * 512), NK)
                dst = s["t"][:, oc, nh * 512 : (nh + 1) * 512]
                if need_bias:
                    nc.scalar.activation(dst, ps[:], Act.Identity, bias=c1[:, oc : oc + 1])
                else:
                    nc.scalar.copy(dst, ps[:])

            def ut_mm(b, mb):
                # ut[:, mb, :] = (xn^T M2^T)[mb-block, :]
                s = st[b]
                if "ut" not in s:
                    s["ut"] = wp.tile([128, NBLK, C], CDT, name=f"ut{b}", tag="ut", bufs=2)
                ps = pp.tile([128, 512], F32, name=f"pu{b}_{mb}", tag="mm", bufs=5)
                mm_k(ps,
                     lambda k: ksl(s["xn"], k, mb * 128, (mb + 1) * 128),
                     lambda k: ksl(m2, k, 0, C), NK)
                nc.scalar.copy(s["ut"][:, mb, :], ps[:])

            def rx_mm(b):
                # general-bias path: rx_t[p, mb] = sum_c wr[c] xn[c, m]; the
                # per-key exp bias is INVSQ*rx - SHIFT (+ bq.bk const).
                s = st[b]
                s["rxb"] = wp.tile([128, NBLK], F32, name=f"rxb{b}", tag="rxb", bufs=2)
                for mb in range(NBLK):
                    ps = pp.tile([128, 1], F32, name=f"prx{b}_{mb}", tag="gbc", bufs=1)
                    mm_k(ps,
                         lambda k: ksl(s["xn"], k, mb * 128, (mb + 1) * 128),
                         lambda k: ksl(wr, k, 0, 1), NK)
                    nc.vector.tensor_scalar(
                        out=s["rxb"][:, mb : mb + 1], in0=ps[:],
                        scalar1=INVSQ, scalar2=RXCONST[0] - SHIFT,
                        op0=Alu.mult, op1=Alu.add,
                    )

            def sc_mm(b, mb, nh):
                # scores^T tile [m-block, n-half] + exp -> E fp8
                s = st[b]
                if "E" not in s:
                    s["E"] = wp.tile([128, NBLK, HW], CDT, name=f"E{b}", tag="E", bufs=2)
                ps = pp.tile([128, 512], F32, name=f"psc{b}_{mb}_{nh}", tag="mm", bufs=5)
                mm_k(ps,
                     lambda k: ksl(s["t"], k, mb * 128, (mb + 1) * 128),
                     lambda k: ksl(s["xn"], k, nh * 512, (nh + 1) * 512), NK)
                bias = s["rxb"][:, mb : mb + 1] if need_bias else ebias[:, 0:1]
                nc.scalar.activation(
                    s["E"][:, mb, nh * 512 : (nh + 1) * 512], ps[:],
                    Act.Exp, bias=bias, scale=INVSQ,
                )

            def z_mm(b, nh):
                # Z[n] = sum_m E[m, n] via ones-stationary matmul; broadcast
                # across partitions; approx-reciprocal (18 bits, plenty).
                s = st[b]
                if "zr" not in s:
                    s["zr"] = wp.tile([128, 2, 512], F32, name=f"zr{b}", tag="zr", bufs=2)
                    s["zsb"] = wp.tile([1, 2, 512], BF16, name=f"zsb{b}", tag="zsb", bufs=2)
                psZ = pp.tile([1, 512], F32, name=f"psz{b}_{nh}", tag="z", bufs=1)
                mm_k(psZ,
                     lambda k: ones_dr[:, :, 0:1] if FP8 else ones_dr[:, 0, 0:1],
                     lambda k: ksl(s["E"], k, nh * 512, (nh + 1) * 512), NM)
                nc.vector.tensor_copy(s["zsb"][:, nh, :], psZ[:])
                psB = pp.tile([128, 512], F32, name=f"psb{b}_{nh}", tag="mm", bufs=5)
                nc.tensor.matmul(psB[:], ones_bc[:], s["zsb"][:, nh, :], start=True, stop=True)
                zb = wp.tile([128, 512], F32, name=f"zb{b}_{nh}", tag="zb", bufs=2)
                nc.scalar.copy(zb[:], psB[:])
                nc.vector.reciprocal(s["zr"][:, nh, :], zb[:])

            def o_mm(b, ct, nh):
                # out[ct-chunk, nh-half] = outU * zr (+ b_f) + skip, streamed out
                s = st[b]
                out_r = out_d[b].rearrange("(ct p) n -> p ct n", p=128)
                ps = pp.tile([128, 512], F32, name=f"po{b}_{ct}_{nh}", tag="mm", bufs=5)
                mm_k(ps,
                     lambda k: ksl(s["ut"], k, ct * 128, (ct + 1) * 128),
                     lambda k: ksl(s["E"], k, nh * 512, (nh + 1) * 512), NM)
                tmp = wp.tile([128, 512], F32, name=f"tmp{b}_{ct}_{nh}", tag="tmp", bufs=4)
                nc.vector.tensor_tensor(tmp[:], ps[:], s["zr"][:, nh, :], op=Alu.mult)
                sl = s["x"][:, ct, nh * 512 : (nh + 1) * 512]
                if need_bias:
                    nc.vector.scalar_tensor_tensor(
                        out=sl, in0=tmp[:], scalar=b_f[:, ct : ct + 1], in1=sl,
                        op0=Alu.add, op1=Alu.add,
                    )
                else:
                    nc.gpsimd.tensor_tensor(sl, tmp[:], sl, op=Alu.add)
                nc.sync.dma_start(out_r[:, ct, nh * 512 : (nh + 1) * 512], sl)

            # ---- emission order == scheduler priority ----
            load_x(0)
            m1 = cp.tile([128, CT, C], CDT, name="m1", tag="m1")
            nc.sync.dma_start(m1[:], m1_d.rearrange("(kc p) o -> p kc o", p=128))
            m2 = cp.tile([128, CT, C], CDT, name="m2", tag="m2")
            nc.sync.dma_start(m2[:], m2_d.rearrange("(kc p) o -> p kc o", p=128))
            load_x(1)

            gn_stats(0)
            gn_chain(0)
            gn_stats(1)
            if need_bias:
                rx_mm(0)
            for oc in range(CT):
                t_mm(0, oc, 0); t_mm(0, oc, 1)
            for mb in range(NBLK):
                ut_mm(0, mb)
            gn_chain(1)
            if need_bias:
                rx_mm(1)
            for mb in range(NBLK):
                sc_mm(0, mb, 0)
            for mb in range(NBLK):
                sc_mm(0, mb, 1)
            for oc in range(CT):
                t_mm(1, oc, 0); t_mm(1, oc, 1)
            z_mm(0, 0)
            for ct in range(CT):
                o_mm(0, ct, 0)
            for mb in range(NBLK):
                ut_mm(1, mb)
            z_mm(0, 1)
            for ct in range(CT):
                o_mm(0, ct, 1)
            for mb in range(NBLK):
                sc_mm(1, mb, 0)
            for mb in range(NBLK):
                sc_mm(1, mb, 1)
            z_mm(1, 0)
            for ct in range(CT):
                o_mm(1, ct, 0)
            z_mm(1, 1)
            for ct in range(CT):
                o_mm(1, ct, 1)

    nc.compile()
    return nc


# constant exp-bias addend for the general-bias path (bq.bk term);
# set by _make_in_maps before the program is built
RXCONST = [0.0]


def _get_program(need_bias):
    key = (V2DT, SHIFT, need_bias)
    if key not in _CACHE:
        _CACHE[key] = _build_program(need_bias)
    return _CACHE[key]


def _to_compute(a):
    """Convert host fp32 weights to the matmul compute format."""
    import ml_dtypes
    a = np.ascontiguousarray(a, dtype=np.float32)
    if V2DT == "fp8":
        return np.ascontiguousarray(a.astype(ml_dtypes.float8_e4m3))
    return np.ascontiguousarray(a.astype(ml_dtypes.bfloat16))


def _make_in_maps(x, gamma, beta, w_in, b_in, w_out, b_out):
    x = np.ascontiguousarray(x.reshape(B, C, HW), dtype=np.float32)
    w_in = np.asarray(w_in, dtype=np.float32)
    w_out = np.asarray(w_out, dtype=np.float32)
    b_in = np.asarray(b_in, dtype=np.float32)
    b_out = np.asarray(b_out, dtype=np.float32)
    wq, wk, wv = w_in[0:C], w_in[C : 2 * C], w_in[2 * C : 3 * C]
    bq, bk, bv = b_in[0:C], b_in[C : 2 * C], b_in[2 * C : 3 * C]
    m1 = wq.T @ wk                      # scores = xn^T m1 xn (+ bias terms)
    m2 = w_out @ wv                     # out = m2 xn attn^T + bf
    c1 = wq.T @ bk                      # q-side bias fold (per-channel)
    bf = w_out @ bv + b_out             # exact: softmax rows sum to 1
    wr = wk.T @ bq                      # k-side bias: varies along keys m
    need_bias = bool(np.any(c1) or np.any(bf) or np.any(wr) or np.any(bq))
    RXCONST[0] = float(INVSQ * np.dot(bq, bk))

    def cvec(v):
        return np.ascontiguousarray(v.reshape(CT, 128).T, dtype=np.float32)

    consts = {
        "m1t": _to_compute(m1.T),
        "m2t": _to_compute(m2.T),
        "gamma_t": cvec(np.asarray(gamma, dtype=np.float32)),
        "beta_t": cvec(np.asarray(beta, dtype=np.float32)),
        "ind_dn": (np.arange(128)[:, None] // GSIZE == np.arange(GSLOT)[None, :]).astype(np.float32),
        "ind_up": (np.arange(GSLOT)[:, None] == np.arange(128)[None, :] // GSIZE).astype(np.float32),
    }
    if need_bias:
        consts["c1_t"] = cvec(c1)
        consts["bf_t"] = cvec(bf)
        consts["wr_t"] = _to_compute(wr.reshape(CT, 128).T)
    return need_bias, [
        {"x": x[c * BL : (c + 1) * BL], **consts}
        for c in range(NCORES)
    ]


def run(inputs, trace=False):
    """Run on 8 cores; returns (output [B,C,H,W], BassKernelResults)."""
    from concourse.bass_utils import run_bass_kernel_spmd

    need_bias, in_maps = _make_in_maps(**inputs)
    nc = _get_program(need_bias)
    res = run_bass_kernel_spmd(nc, in_maps, core_ids=list(range(NCORES)), trace=trace)
    out = np.concatenate([res.results[i]["out"] for i in range(NCORES)], axis=0)
    return out.reshape(B, C, H, W).astype(np.float32), res


def kernel(**inputs) -> np.ndarray:
    out, _ = run(inputs)
    return out


# revision 12
# speedup vs baseline: 1.1715x; 1.1715x over previous
"""AttentionBlock (GroupNorm + 1x1-conv QKV + spatial attention + 1x1-conv out
+ skip) on 8 Trainium2 NeuronCores.

Sharding: data-parallel over batch. B=16 -> 2 batches per core, weights
replicated, no collectives. Each core runs the same NEFF on its own batch
slice; the host gathers by concatenation.

v3 design (vs the q/k/v baseline):
  * Host folds the 1x1 convs:  M1 = W_q^T W_k  and  M2 = W_o W_v, so
        scores = xn^T M1 xn          (one projection t = M1 xn instead of q,k)
        out    = (M2 xn) attn^T + (W_o b_v + b_o)   (no separate v / proj_out)
    The bias fold is exact because softmax rows sum to 1.
  * Scores are computed TRANSPOSED ([key m on partitions, query n free]):
    kills all PE transposes and the attn normalization pass. The softmax
    denominator Z comes from a ones-stationary matmul over exp(scores^T);
    normalization happens once, fused into the output evacuation
    (out = outU * (1/Z)[n] + skip).
  * All five big matmul groups run fp8e4 DoubleRow (2 MACs/cell/cycle);
    exp() is biased into fp8 range (the fixed e^-SHIFT cancels in Z).
  * Both batches' GroupNorm runs up front (vectorized across channel
    chunks) so the ACT table only swaps once into Exp and the PE never
    stalls mid-kernel on a norm chain.
  * Engine budget: PE ~53us; ACT = exp + all PSUM->SBUF copies (fast at
    ~250ns/tile); DVE = reduces + zr chain + out normalize; Pool = GN
    apply + skip-add.

Layouts on chip (partition dim first):
  channels  c = 128*ct + p   (ct in 0..3)
  spatial   n = 128*mb + p   (mb in 0..7)
  x             [128, 4, 1024]  f32   ([c_part, ct, n])
  xn, t         [128, 4, 1024]  fp8   (t indexed [c_q, m])
  ut            [128, 8, 512]   fp8   ([m_part, mb, c_out])
  E = exp(S^T)  [128, 8, 1024]  fp8   ([m_part, mb, n])
  zr = 1/Z      [128, 2, 512]   f32   (Z broadcast over partitions)
"""

import os
import numpy as np

B, C, H, W = 16, 512, 32, 32
HW = H * W            # 1024
BL = 2                # batches per core
NCORES = 8
CT = C // 128         # 4 channel chunks
NBLK = HW // 128      # 8 spatial blocks
GSIZE = 16            # channels per group
GSLOT = 128 // GSIZE  # 8 groups per channel chunk
CNT = GSIZE * HW      # elements per group (16384)
EPS = 1e-5
INVSQ = float(1.0 / np.sqrt(np.float32(C)))
# exp(score/sqrt(C) - SHIFT): keeps exp output in fp8e4m3's range
# (max |score/sqrt(C)| ~ 6.2 -> exp <= ~110 < 240). e^-SHIFT cancels in Z.
SHIFT = float(os.environ.get("K_SHIFT", "1.5"))
# "fp8": DoubleRow fp8 for all big matmuls. "bf16": same structure, bf16.
V2DT = os.environ.get("K_V2DT", "fp8")
NWARM = int(os.environ.get("K_NWARM", "8"))

_CACHE = {}


def _build_program(need_bias):
    import concourse.bacc as bacc
    import concourse.tile as tile
    from concourse import mybir

    F32 = mybir.dt.float32
    Alu = mybir.AluOpType
    Act = mybir.ActivationFunctionType
    Ax = mybir.AxisListType
    BF16 = mybir.dt.bfloat16
    FP8 = V2DT == "fp8"
    CDT = mybir.dt.float8e4 if FP8 else BF16
    DR = mybir.MatmulPerfMode.DoubleRow if FP8 else None
    KSTEP = 2 if FP8 else 1   # kc contraction per matmul (DoubleRow pairs)
    NK = CT // KSTEP
    NM = NBLK // KSTEP

    nc = bacc.Bacc("TRN2", target_bir_lowering=False, debug=False)

    x_d = nc.dram_tensor("x", [BL, C, HW], F32, kind="ExternalInput")
    m1_d = nc.dram_tensor("m1t", [C, C], CDT, kind="ExternalInput")
    m2_d = nc.dram_tensor("m2t", [C, C], CDT, kind="ExternalInput")
    gam_d = nc.dram_tensor("gamma_t", [128, CT], F32, kind="ExternalInput")
    bet_d = nc.dram_tensor("beta_t", [128, CT], F32, kind="ExternalInput")
    idn_dn_d = nc.dram_tensor("ind_dn", [128, GSLOT], F32, kind="ExternalInput")
    idn_up_d = nc.dram_tensor("ind_up", [GSLOT, 128], F32, kind="ExternalInput")
    if need_bias:
        c1_d = nc.dram_tensor("c1_t", [128, CT], F32, kind="ExternalInput")
        bf_d = nc.dram_tensor("bf_t", [128, CT], F32, kind="ExternalInput")
        wr_d = nc.dram_tensor("wr_t", [128, CT], CDT, kind="ExternalInput")
    out_d = nc.dram_tensor("out", [BL, C, HW], F32, kind="ExternalOutput")

    with tile.TileContext(nc) as tc:
        with (
            tc.tile_pool(name="consts", bufs=1) as cp,
            tc.tile_pool(name="work", bufs=1) as wp,
            tc.tile_pool(name="psum", bufs=1, space="PSUM") as pp,
        ):
            # ---- PE warm-up: the HAM clock gate holds the PE at 1.2 GHz
            # until it sees ~3.4us of sustained matmul activity. Burn the
            # DMA + GroupNorm lead-in on throwaway matmuls.
            warm = cp.tile([128, 512], BF16, name="warm", tag="warm")
            nc.gpsimd.memset(warm[:], 1.0)
            warm_ps = pp.tile([128, 512], F32, name="warm_ps", tag="warm_ps", bufs=1)
            for _ in range(NWARM):
                nc.tensor.matmul(warm_ps[:], warm[:, 0:128], warm[:], start=True, stop=True)

            # ---- small constants on the SWDGE queue ----
            ind_dn = cp.tile([128, GSLOT], F32, name="ind_dn", tag="ind_dn")
            nc.gpsimd.dma_start(ind_dn[:], idn_dn_d[:])
            ind_up = cp.tile([GSLOT, 128], F32, name="ind_up", tag="ind_up")
            nc.gpsimd.dma_start(ind_up[:], idn_up_d[:])
            gam = cp.tile([128, CT], F32, name="gam", tag="gam")
            nc.gpsimd.dma_start(gam[:], gam_d[:])
            bet = cp.tile([128, CT], F32, name="bet", tag="bet")
            nc.gpsimd.dma_start(bet[:], bet_d[:])
            if need_bias:
                c1 = cp.tile([128, CT], F32, name="c1", tag="c1")
                nc.gpsimd.dma_start(c1[:], c1_d[:])
                b_f = cp.tile([128, CT], F32, name="b_f", tag="b_f")
                nc.gpsimd.dma_start(b_f[:], bf_d[:])
                wr = cp.tile([128, CT], CDT, name="wr", tag="wr")
                nc.gpsimd.dma_start(wr[:], wr_d[:])
            ones_dr = cp.tile([128, KSTEP, 16], CDT, name="ones_dr", tag="ones_dr")
            nc.gpsimd.memset(ones_dr[:], 1.0)
            ones_bc = cp.tile([1, 128], BF16, name="ones_bc", tag="ones_bc")
            nc.gpsimd.memset(ones_bc[:], 1.0)
            ebias = cp.tile([128, 1], F32, name="ebias", tag="ebias")
            nc.gpsimd.memset(ebias[:], -SHIFT)

            st = [dict() for _ in range(BL)]

            def load_x(b):
                s = st[b]
                s["x"] = wp.tile([128, CT, HW], F32, name=f"x{b}", tag="x", bufs=2)
                x_r = x_d[b].rearrange("(ct p) n -> p ct n", p=128)
                for ct in range(CT):
                    nc.sync.dma_start(s["x"][:, ct, :], x_r[:, ct, :])

            def gn_stats(b, ct):
                # per-partition sum (DVE) and sum of squares: ACT Square for
                # batch 0 (ACT idle early), DVE scalar_tensor_tensor+accum for
                # batch 1 (keeps the ACT FIFO free for batch-0 evacuations).
                s = st[b]
                if "ssum" not in s:
                    s["ssum"] = wp.tile([128, 2 * CT], F32, name=f"ssum{b}", tag="ssum", bufs=2)
                    s["scr"] = wp.tile([128, HW], F32, name=f"scr{b}", tag="scr", bufs=2)
                nc.vector.tensor_reduce(
                    out=s["ssum"][:, 2 * ct : 2 * ct + 1], in_=s["x"][:, ct, :],
                    axis=Ax.X, op=Alu.add,
                )
                if b == 0:
                    nc.scalar.activation(
                        s["scr"][:], s["x"][:, ct, :], Act.Square,
                        accum_out=s["ssum"][:, 2 * ct + 1 : 2 * ct + 2],
                    )
                else:
                    nc.vector.scalar_tensor_tensor(
                        out=s["scr"][:], in0=s["x"][:, ct, :], scalar=0.0,
                        in1=s["x"][:, ct, :], op0=Alu.add, op1=Alu.mult,
                        accum_out=s["ssum"][:, 2 * ct + 1 : 2 * ct + 2],
                    )

            def gn_chain(b, ct):
                # group sums across partitions (tiny PE matmul against the
                # indicator), mean/rstd chain, broadcast back, fused apply
                # on Pool writing xn in the compute dtype
                s = st[b]
                if "xn" not in s:
                    s["xn"] = wp.tile([128, CT, HW], CDT, name=f"xn{b}", tag="xn", bufs=2)
                    s["ab"] = wp.tile([128, 2 * CT], F32, name=f"ab{b}", tag="ab", bufs=2)
                ab = s["ab"]
                ps_g = pp.tile([GSLOT, 2], F32, name=f"psg{b}_{ct}", tag="gbc", bufs=1)
                nc.tensor.matmul(ps_g[:], ind_dn[:], s["ssum"][:, 2 * ct : 2 * ct + 2], start=True, stop=True)
                m_r = wp.tile([GSLOT, 2], F32, name=f"mr{b}_{ct}", tag="mr", bufs=4)
                t2 = wp.tile([GSLOT, 2], F32, name=f"t2{b}_{ct}", tag="t2", bufs=4)
                nc.scalar.mul(m_r[:, 0:1], ps_g[:, 0:1], 1.0 / CNT)     # mean
                nc.scalar.mul(t2[:, 0:1], ps_g[:, 1:2], 1.0 / CNT)      # E[x^2]
                nc.vector.tensor_mul(t2[:, 1:2], m_r[:, 0:1], m_r[:, 0:1])
                nc.vector.tensor_sub(t2[:, 0:1], t2[:, 0:1], t2[:, 1:2])
                nc.vector.tensor_scalar_add(t2[:, 0:1], t2[:, 0:1], EPS)
                nc.scalar.activation(t2[:, 0:1], t2[:, 0:1], Act.Sqrt)
                nc.vector.reciprocal(m_r[:, 1:2], t2[:, 0:1])           # rstd
                ps_bc = pp.tile([128, 2], F32, name=f"psbc{b}_{ct}", tag="gbc", bufs=1)
                nc.tensor.matmul(ps_bc[:], ind_up[:], m_r[:], start=True, stop=True)
                nc.vector.tensor_mul(ab[:, ct : ct + 1], ps_bc[:, 1:2], gam[:, ct : ct + 1])
                nc.vector.tensor_mul(ab[:, CT + ct : CT + ct + 1], ps_bc[:, 0:1], ab[:, ct : ct + 1])
                nc.vector.tensor_sub(ab[:, CT + ct : CT + ct + 1], bet[:, ct : ct + 1], ab[:, CT + ct : CT + ct + 1])
                nc.gpsimd.tensor_scalar(
                    out=s["xn"][:, ct, :], in0=s["x"][:, ct, :],
                    scalar1=ab[:, ct : ct + 1], scalar2=ab[:, CT + ct : CT + ct + 1],
                    op0=Alu.mult, op1=Alu.add,
                )

            def mm_k(ps, lhs_fn, rhs_fn, nk):
                for k in range(nk):
                    nc.tensor.matmul(
                        ps[:], lhs_fn(k), rhs_fn(k),
                        start=(k == 0), stop=(k == nk - 1),
                        perf_mode=DR,
                    )

            def ksl(t, k, lo, hi):
                return t[:, KSTEP * k : KSTEP * (k + 1), lo:hi]

            def t_mm(b, oc, nh):
                # t[:, oc, nh-half] = (M1 xn)[oc-chunk, half]  (+ c1 if biased)
                s = st[b]
                if "t" not in s:
                    s["t"] = wp.tile([128, CT, HW], CDT, name=f"t{b}", tag="t", bufs=2)
                ps = pp.tile([128, 512], F32, name=f"pt{b}_{oc}_{nh}", tag="mm", bufs=5)
                mm_k(ps,
                     lambda k: ksl(m1, k, oc * 128, (oc + 1) * 128),
                     lambda k: ksl(s["xn"], k, nh * 512, (nh + 1) * 512), NK)
                dst = s["t"][:, oc, nh * 512 : (nh + 1) * 512]
                if need_bias:
                    nc.scalar.activation(dst, ps[:], Act.Identity, bias=c1[:, oc : oc + 1])
                else:
                    nc.vector.tensor_copy(dst, ps[:])

            def ut_mm(b, mb):
                # ut[:, mb, :] = (xn^T M2^T)[mb-block, :]
                s = st[b]
                if "ut" not in s:
                    s["ut"] = wp.tile([128, NBLK, C], CDT, name=f"ut{b}", tag="ut", bufs=2)
                ps = pp.tile([128, 512], F32, name=f"pu{b}_{mb}", tag="mm", bufs=5)
                mm_k(ps,
                     lambda k: ksl(s["xn"], k, mb * 128, (mb + 1) * 128),
                     lambda k: ksl(m2, k, 0, C), NK)
                nc.scalar.copy(s["ut"][:, mb, :], ps[:])

            def rx_mm(b):
                # general-bias path: rx_t[p, mb] = sum_c wr[c] xn[c, m]; the
                # per-key exp bias is INVSQ*rx - SHIFT (+ bq.bk const).
                s = st[b]
                s["rxb"] = wp.tile([128, NBLK], F32, name=f"rxb{b}", tag="rxb", bufs=2)
                for mb in range(NBLK):
                    ps = pp.tile([128, 1], F32, name=f"prx{b}_{mb}", tag="gbc", bufs=1)
                    mm_k(ps,
                         lambda k: ksl(s["xn"], k, mb * 128, (mb + 1) * 128),
                         lambda k: ksl(wr, k, 0, 1), NK)
                    nc.vector.tensor_scalar(
                        out=s["rxb"][:, mb : mb + 1], in0=ps[:],
                        scalar1=INVSQ, scalar2=RXCONST[0] - SHIFT,
                        op0=Alu.mult, op1=Alu.add,
                    )

            def sc_mm(b, mb, nh):
                # scores^T tile [m-block, n-half] + exp -> E fp8
                s = st[b]
                if "E" not in s:
                    s["E"] = wp.tile([128, NBLK, HW], CDT, name=f"E{b}", tag="E", bufs=2)
                ps = pp.tile([128, 512], F32, name=f"psc{b}_{mb}_{nh}", tag="mm", bufs=5)
                mm_k(ps,
                     lambda k: ksl(s["t"], k, mb * 128, (mb + 1) * 128),
                     lambda k: ksl(s["xn"], k, nh * 512, (nh + 1) * 512), NK)
                bias = s["rxb"][:, mb : mb + 1] if need_bias else ebias[:, 0:1]
                nc.scalar.activation(
                    s["E"][:, mb, nh * 512 : (nh + 1) * 512], ps[:],
                    Act.Exp, bias=bias, scale=INVSQ,
                )

            def z_mm(b, nh):
                # Z[n] = sum_m E[m, n] via ones-stationary matmul; broadcast
                # across partitions; approx-reciprocal (18 bits, plenty).
                s = st[b]
                if "zr" not in s:
                    s["zr"] = wp.tile([128, 2, 512], F32, name=f"zr{b}", tag="zr", bufs=2)
                    s["zsb"] = wp.tile([1, 2, 512], BF16, name=f"zsb{b}", tag="zsb", bufs=2)
                psZ = pp.tile([1, 512], F32, name=f"psz{b}_{nh}", tag="z", bufs=1)
                mm_k(psZ,
                     lambda k: ones_dr[:, :, 0:1] if FP8 else ones_dr[:, 0, 0:1],
                     lambda k: ksl(s["E"], k, nh * 512, (nh + 1) * 512), NM)
                nc.vector.tensor_copy(s["zsb"][:, nh, :], psZ[:])
                psB = pp.tile([128, 512], F32, name=f"psb{b}_{nh}", tag="mm", bufs=5)
                nc.tensor.matmul(psB[:], ones_bc[:], s["zsb"][:, nh, :], start=True, stop=True)
                nc.vector.reciprocal_approx_fast(s["zr"][:, nh, :], psB[:])

            def o_mm(b, ct, nh):
                # out[ct-chunk, nh-half] = outU * zr (+ b_f) + skip, streamed out
                s = st[b]
                out_r = out_d[b].rearrange("(ct p) n -> p ct n", p=128)
                ps = pp.tile([128, 512], F32, name=f"po{b}_{ct}_{nh}", tag="mm", bufs=5)
                mm_k(ps,
                     lambda k: ksl(s["ut"], k, ct * 128, (ct + 1) * 128),
                     lambda k: ksl(s["E"], k, nh * 512, (nh + 1) * 512), NM)
                tmp = wp.tile([128, 512], F32, name=f"tmp{b}_{ct}_{nh}", tag="tmp", bufs=4)
                nc.vector.tensor_tensor(tmp[:], ps[:], s["zr"][:, nh, :], op=Alu.mult)
                sl = s["x"][:, ct, nh * 512 : (nh + 1) * 512]
                if need_bias:
                    nc.vector.scalar_tensor_tensor(
                        out=sl, in0=tmp[:], scalar=b_f[:, ct : ct + 1], in1=sl,
                        op0=Alu.add, op1=Alu.add,
                    )
                else:
                    nc.gpsimd.tensor_tensor(sl, tmp[:], sl, op=Alu.add)
                nc.sync.dma_start(out_r[:, ct, nh * 512 : (nh + 1) * 512], sl)

            # ---- emission order == scheduler priority ----
            s0 = st[0]
            s0["x"] = wp.tile([128, CT, HW], F32, name="x0", tag="x", bufs=2)
            x_r0 = x_d[0].rearrange("(ct p) n -> p ct n", p=128)
            nc.sync.dma_start(s0["x"][:, 0, :], x_r0[:, 0, :])
            nc.sync.dma_start(s0["x"][:, 1, :], x_r0[:, 1, :])
            m1 = cp.tile([128, CT, C], CDT, name="m1", tag="m1")
            nc.sync.dma_start(m1[:], m1_d.rearrange("(kc p) o -> p kc o", p=128))
            nc.sync.dma_start(s0["x"][:, 2, :], x_r0[:, 2, :])
            nc.sync.dma_start(s0["x"][:, 3, :], x_r0[:, 3, :])
            m2 = cp.tile([128, CT, C], CDT, name="m2", tag="m2")
            nc.sync.dma_start(m2[:], m2_d.rearrange("(kc p) o -> p kc o", p=128))
            load_x(1)

            for ct in range(CT):
                gn_stats(0, ct)
                gn_chain(0, ct)
            if need_bias:
                rx_mm(0)
            for oc in range(CT):
                t_mm(0, oc, 0); t_mm(0, oc, 1)
            for mb in range(NBLK):
                ut_mm(0, mb)
            for ct in range(CT):
                gn_stats(1, ct)
                gn_chain(1, ct)
            if need_bias:
                rx_mm(1)
            for mb in range(NBLK):
                sc_mm(0, mb, 0)
            for mb in range(NBLK):
                sc_mm(0, mb, 1)
            for oc in range(CT):
                t_mm(1, oc, 0); t_mm(1, oc, 1)
            z_mm(0, 0)
            for ct in range(CT):
                o_mm(0, ct, 0)
            for mb in range(NBLK):
                ut_mm(1, mb)
            z_mm(0, 1)
            for ct in range(CT):
                o_mm(0, ct, 1)
            for mb in range(NBLK):
                sc_mm(1, mb, 0)
            for mb in range(NBLK):
                sc_mm(1, mb, 1)
            z_mm(1, 0)
            for ct in range(CT):
                o_mm(1, ct, 0)
            z_mm(1, 1)
            for ct in range(CT):
                o_mm(1, ct, 1)

    nc.compile()
    return nc


# constant exp-bias addend for the general-bias path (bq.bk term);
# set by _make_in_maps before the program is built
RXCONST = [0.0]


def _get_program(need_bias):
    key = (V2DT, SHIFT, need_bias)
    if key not in _CACHE:
        _CACHE[key] = _build_program(need_bias)
    return _CACHE[key]


def _to_compute(a):
    """Convert host fp32 weights to the matmul compute format."""
    import ml_dtypes
    a = np.ascontiguousarray(a, dtype=np.float32)
    if V2DT == "fp8":
        return np.ascontiguousarray(a.astype(ml_dtypes.float8_e4m3))
    return np.ascontiguousarray(a.astype(ml_dtypes.bfloat16))


def _make_in_maps(x, gamma, beta, w_in, b_in, w_out, b_out):
    x = np.ascontiguousarray(x.reshape(B, C, HW), dtype=np.float32)
    w_in = np.asarray(w_in, dtype=np.float32)
    w_out = np.asarray(w_out, dtype=np.float32)
    b_in = np.asarray(b_in, dtype=np.float32)
    b_out = np.asarray(b_out, dtype=np.float32)
    wq, wk, wv = w_in[0:C], w_in[C : 2 * C], w_in[2 * C : 3 * C]
    bq, bk, bv = b_in[0:C], b_in[C : 2 * C], b_in[2 * C : 3 * C]
    m1 = wq.T @ wk                      # scores = xn^T m1 xn (+ bias terms)
    m2 = w_out @ wv                     # out = m2 xn attn^T + bf
    c1 = wq.T @ bk                      # q-side bias fold (per-channel)
    bf = w_out @ bv + b_out             # exact: softmax rows sum to 1
    wr = wk.T @ bq                      # k-side bias: varies along keys m
    need_bias = bool(np.any(c1) or np.any(bf) or np.any(wr) or np.any(bq))
    RXCONST[0] = float(INVSQ * np.dot(bq, bk))

    def cvec(v):
        return np.ascontiguousarray(v.reshape(CT, 128).T, dtype=np.float32)

    consts = {
        "m1t": _to_compute(m1.T),
        "m2t": _to_compute(m2.T),
        "gamma_t": cvec(np.asarray(gamma, dtype=np.float32)),
        "beta_t": cvec(np.asarray(beta, dtype=np.float32)),
        "ind_dn": (np.arange(128)[:, None] // GSIZE == np.arange(GSLOT)[None, :]).astype(np.float32),
        "ind_up": (np.arange(GSLOT)[:, None] == np.arange(128)[None, :] // GSIZE).astype(np.float32),
    }
    if need_bias:
        consts["c1_t"] = cvec(c1)
        consts["bf_t"] = cvec(bf)
        consts["wr_t"] = _to_compute(wr.reshape(CT, 128).T)
    return need_bias, [
        {"x": x[c * BL : (c + 1) * BL], **consts}
        for c in range(NCORES)
    ]


def run(inputs, trace=False):
    """Run on 8 cores; returns (output [B,C,H,W], BassKernelResults)."""
    from concourse.bass_utils import run_bass_kernel_spmd

    need_bias, in_maps = _make_in_maps(**inputs)
    nc = _get_program(need_bias)
    res = run_bass_kernel_spmd(nc, in_maps, core_ids=list(range(NCORES)), trace=trace)
    out = np.concatenate([res.results[i]["out"] for i in range(NCORES)], axis=0)
    return out.reshape(B, C, H, W).astype(np.float32), res


def kernel(**inputs) -> np.ndarray:
    out, _ = run(inputs)
    return out
